# revision 1
# baseline (speedup 1.0000x reference)
"""EUNN cell (B=2048, H=1024, capacity=128) on 8 NeuronCores.

Strategy: the 128 Givens-rotation layers compose into a banded complex matrix
M = L_128...L_1 with bandwidth <= 128, i.e. block-tridiagonal in 128-blocks.
The tiny parameter preprocessing composes M on host (numpy, ~100 MFLOP);
the device kernel is the bandwidth-heavy part: out^T = (D_omega M) x^T as
fp16 TensorEngine matmuls with fp32 PSUM accumulation.

Sharding: 8 cores = 4 batch quarters x 2 hidden halves. Each core computes
out^T rows [4j*128, 4j*128+512) for batch columns [i*512, (i+1)*512):
48 matmuls of [K=128]x[N=512], 8 PSUM banks, ~90 instructions total.
"""
import numpy as np

H = 1024
B = 2048
CAP = 128
EH = H // 2
OH = (H - 1) // 2
EC = (CAP + 1) // 2
OC = CAP // 2
BAND = CAP
NC_CORES = 8
NB = H // 128          # 8 hidden blocks
NJ = 2                 # hidden halves
NI = 4                 # batch quarters
BCORE = B // NI        # 512 batch cols per core
RH = NB // NJ          # 4 r-blocks per core
CS = RH + 2            # 6 c-blocks per core slab (with halo + dummy pad)
NPAIR = RH * 3         # 12 (r, c) block pairs per core (some zero-padded)

_perm_even = np.arange(EH * 2).reshape(-1, 2)[:, ::-1].reshape(-1)
_perm_odd = np.concatenate(
    [[0], np.arange(1, OH * 2 + 1).reshape(-1, 2)[:, ::-1].reshape(-1), [OH * 2 + 1]]
)


def _interleave(a, b):
    return np.stack([a, b], axis=-1).reshape(-1)


def _layer_coeffs(even_theta, odd_theta, even_phi, odd_phi):
    ce, se = np.cos(even_theta), np.sin(even_theta)
    cpe, spe = np.cos(even_phi), np.sin(even_phi)
    co, so = np.cos(odd_theta), np.sin(odd_theta)
    cpo, spo = np.cos(odd_phi), np.sin(odd_phi)
    zE = np.zeros(EH)
    zO = np.zeros(OH)
    one = np.ones(1)
    zero = np.zeros(1)
    for t in range(EC):
        ect, est, ecp, esp = ce[t], se[t], cpe[t], spe[t]
        v1 = _interleave(esp * ect, ect) + 1j * _interleave(ecp * ect, zE)
        v2 = _interleave(-esp * est, est) + 1j * _interleave(-ecp * est, zE)
        yield v1, v2, _perm_even
        oct_, ost, ocp, osp = co[t], so[t], cpo[t], spo[t]
        v1 = np.concatenate([one, _interleave(osp * oct_, oct_), one]) + 1j * np.concatenate(
            [zero, _interleave(ocp * oct_, zO), zero]
        )
        v2 = np.concatenate([zero, _interleave(-osp * ost, ost), zero]) + 1j * np.concatenate(
            [zero, _interleave(-ocp * ost, zO), zero]
        )
        yield v1, v2, _perm_odd


def _compose_banded(even_theta, odd_theta, even_phi, odd_phi):
    """M = L_128...L_1 as band array bnd[i, d], column j = i + d - BAND.

    Layer update: new[i, d] = v1[i]*bnd[i, d] + v2[i]*bnd[perm[i], d - s[i]],
    s[i] = perm[i] - i. Both layer types pair adjacent rows, so the update
    splits into two strided halves with fixed +-1 column shifts.
    """
    W = 2 * BAND + 1
    bnd = np.zeros((H, W), np.complex64)
    bnd[:, BAND] = 1.0
    new = np.zeros_like(bnd)
    for v1, v2, perm in _layer_coeffs(even_theta, odd_theta, even_phi, odd_phi):
        if perm is _perm_even:
            lo, hi = 0, H  # pairs (0,1),(2,3),...
        else:
            lo, hi = 1, H - 1  # pairs (1,2),(3,4),...; rows 0, H-1 fixed
            new[0] = v1[0] * bnd[0]
            new[H - 1] = v1[H - 1] * bnd[H - 1]
        a = bnd[lo:hi:2]      # upper row of each pair (s=+1)
        b = bnd[lo + 1:hi:2]  # lower row of each pair (s=-1)
        v1a = v1[lo:hi:2, None]
        v2a = v2[lo:hi:2, None]
        v1b = v1[lo + 1:hi:2, None]
        v2b = v2[lo + 1:hi:2, None]
        na = new[lo:hi:2]
        nb = new[lo + 1:hi:2]
        # upper: partner is lower row, shifted right in d (d-1)
        np.multiply(v1a, a, out=na)
        na[:, 1:] += (v2a * b[:, :-1]).astype(np.complex64)
        # lower: partner is upper row, shifted left in d (d+1)
        np.multiply(v1b, b, out=nb)
        nb[:, :-1] += (v2b * a[:, 1:]).astype(np.complex64)
        bnd, new = new, bnd
    return bnd


def _banded_to_dense(bnd):
    M = np.zeros((H, H), bnd.dtype)
    rows = np.arange(H)
    for d in range(2 * BAND + 1):
        j = rows + d - BAND
        ok = (j >= 0) & (j < H)
        M[rows[ok], j[ok]] = bnd[ok, d]
    return M


_NC_CACHE = {}


def _build_device_kernel(reps=1):
    key = ("nc", reps)
    if key in _NC_CACHE:
        return _NC_CACHE[key]
    import concourse.tile as tile
    from concourse import bacc, mybir

    f16 = mybir.dt.float16
    f32 = mybir.dt.float32
    nc = bacc.Bacc("TRN2", target_bir_lowering=False, debug=False)
    # x^T slabs, re then im, each 5 REAL hidden blocks; slab slot 0 (the edge
    # pad) is memset on device. Upper-half cores load blocks mirrored so the
    # pad is at slot 0 for every core (uniform NEFF).
    CR = CS - 1  # real blocks per component
    x_d = nc.dram_tensor("x", [2 * CR * 128, BCORE], f16, kind="ExternalInput").ap()
    # packed lhsT blocks: re pair p at [:, p*128:(p+1)*128], then im pairs
    m_d = nc.dram_tensor("m", [128, 2 * NPAIR * 128], f16, kind="ExternalInput").ap()
    # out^T slabs, re then im, each 4 r-blocks x 512 batch cols (f16: values are
    # fp32-accumulated in PSUM, final rounding ~2.4e-4 relative)
    y_d = nc.dram_tensor("y", [2 * RH * 128, BCORE], f16, kind="ExternalOutput").ap()

    x_v = x_d.rearrange("(q p) b -> p q b", p=128)  # q = 2*CS blocks

    with tile.TileContext(nc) as tc:
        with (
            tc.tile_pool(name="mp", bufs=1) as mpool,
            tc.tile_pool(name="xp", bufs=2 if reps > 1 else 1) as xpool,
            tc.tile_pool(name="op", bufs=2 if reps > 1 else 1) as opool,
            tc.tile_pool(name="pp", bufs=1, space="PSUM") as pspool,
        ):
            m_t = mpool.tile([128, 2 * NPAIR * 128], f16, tag="m")

            def msl(p, im):
                off = (im * NPAIR + p) * 128
                return m_t[:, off : off + 128]

            for _rep in range(reps):
                x_t = xpool.tile([128, 2 * CS * BCORE], f16, tag="x")
                xr3 = x_t[:].rearrange("p (q b) -> p q b", q=2 * CS)
                # pad slots: slot 0 (re) and slot CS (im)
                nc.gpsimd.memset(x_t[:, 0:BCORE], 0.0)
                nc.gpsimd.memset(x_t[:, CS * BCORE : (CS + 1) * BCORE], 0.0)
                # x chunks: (sbuf slot, dram block, nblocks) — re lo/hi, im lo/hi
                XCH = (
                    (1, 0, 3), (4, 3, 2),
                    (CS + 1, CR, 3), (CS + 4, CR + 3, 2),
                )
                # interleave M chunks (re/im x lo/hi pair halves) with x chunks
                # so the first matmul group unblocks as early as possible
                for kind, s in (
                    ("m", 0), ("x", 0), ("m", 1), ("x", 2),
                    ("m", 2), ("x", 1), ("m", 3), ("x", 3),
                ):
                    if kind == "m":
                        if _rep == 0:
                            im, lohi = s % 2, s // 2
                            off = (im * NPAIR + lohi * 6) * 128
                            nc.sync.dma_start(
                                m_t[:, off : off + 6 * 128],
                                m_d[:, off : off + 6 * 128],
                            )
                    else:
                        d0, q0, nb = XCH[s]
                        nc.sync.dma_start(
                            xr3[:, d0 : d0 + nb], x_v[:, q0 : q0 + nb]
                        )
                ximn_t = xpool.tile([128, CS * BCORE], f16, tag="ximn")
                half = CS // 2
                for s in range(2):
                    sl = slice(s * half * BCORE, (s + 1) * half * BCORE)
                    nc.vector.tensor_scalar_mul(
                        ximn_t[:, sl], x_t[:, CS * BCORE :][:, sl], -1.0
                    )

                def xre(cl):
                    return x_t[:, cl * BCORE : (cl + 1) * BCORE]

                def xim(cl):
                    return x_t[:, (CS + cl) * BCORE : (CS + cl + 1) * BCORE]

                def ximn(cl):
                    return ximn_t[:, cl * BCORE : (cl + 1) * BCORE]

                o_t = opool.tile([128, 2 * RH * BCORE], f16, tag="o")

                for rl in range(RH):
                    psr = pspool.tile([128, BCORE], f32, tag=f"psr{rl}")
                    psi = pspool.tile([128, BCORE], f32, tag=f"psi{rl}")
                    for k in range(3):
                        cl = rl + k  # slab col block (slab offset = r0 - 1)
                        p = rl * 3 + k
                        first = k == 0
                        last = k == 2
                        nc.tensor.matmul(psr[:], lhsT=msl(p, 0), rhs=xre(cl), start=first, stop=False)
                        nc.tensor.matmul(psi[:], lhsT=msl(p, 0), rhs=xim(cl), start=first, stop=False)
                        nc.tensor.matmul(psi[:], lhsT=msl(p, 1), rhs=xre(cl), start=False, stop=last)
                        nc.tensor.matmul(psr[:], lhsT=msl(p, 1), rhs=ximn(cl), start=False, stop=last)
                    osl_r = slice(rl * BCORE, (rl + 1) * BCORE)
                    osl_i = slice((RH + rl) * BCORE, (RH + rl + 1) * BCORE)
                    # split PSUM->SBUF copies across ScalarE and VectorE
                    if rl % 2 == 0:
                        nc.scalar.copy(o_t[:, osl_r], psr[:])
                        nc.vector.tensor_copy(o_t[:, osl_i], psi[:])
                    else:
                        nc.vector.tensor_copy(o_t[:, osl_r], psr[:])
                        nc.scalar.copy(o_t[:, osl_i], psi[:])
                y_v = y_d.rearrange("(q p) b -> p q b", p=128)
                o_r = o_t[:].rearrange("p (q b) -> p q b", q=2 * RH)
                for s in range(4):
                    nc.sync.dma_start(
                        y_v[:, s * 2 : s * 2 + 2], o_r[:, s * 2 : s * 2 + 2]
                    )
    nc.compile()
    _NC_CACHE[key] = nc
    return nc


def _host_prepare(x_re, x_im, omega, even_theta, odd_theta, even_phi, odd_phi):
    """Compose M, fold omega, build per-core packed inputs."""
    bnd = _compose_banded(
        even_theta.astype(np.float64),
        odd_theta.astype(np.float64),
        even_phi.astype(np.float64),
        odd_phi.astype(np.float64),
    )
    M = _banded_to_dense(bnd)
    w = omega.astype(np.float64)
    Mw = (np.cos(w) + 1j * np.sin(w))[:, None] * M
    Mre = Mw.real.astype(np.float32)
    Mim = Mw.imag.astype(np.float32)

    xreT = np.ascontiguousarray(x_re.T).astype(np.float16)  # [H, B]
    ximT = np.ascontiguousarray(x_im.T).astype(np.float16)

    CR = CS - 1
    in_maps = []
    for core in range(NC_CORES):
        j, i = divmod(core, NI)
        bs = slice(i * BCORE, (i + 1) * BCORE)
        # slab slot s (1..5) holds hidden block: j=0: s-1 ; j=1: 8-s (mirrored
        # so the out-of-range pad block is always slot 0, memset on device).
        if j == 0:
            blocks = list(range(0, CR))          # DRAM q -> block q
            rmap = lambda rl: rl                 # psum slot rl -> out block
            cmap = lambda rl, k: rl + k - 1      # pair (rl,k) -> M column block
        else:
            blocks = [NB - 1 - q for q in range(CR)]
            rmap = lambda rl: NB - 1 - rl
            cmap = lambda rl, k: NB - rl - k

        x_s = np.empty((2 * CR * 128, BCORE), np.float16)
        for q, blk in enumerate(blocks):
            x_s[q * 128 : (q + 1) * 128] = xreT[blk * 128 : (blk + 1) * 128, bs]
            x_s[(CR + q) * 128 : (CR + q + 1) * 128] = ximT[
                blk * 128 : (blk + 1) * 128, bs
            ]

        m_p = np.zeros((128, 2 * NPAIR * 128), np.float16)
        for rl in range(RH):
            r = rmap(rl)
            for k in range(3):
                c = cmap(rl, k)
                if not (0 <= c < NB):
                    continue  # leave zero block
                p = rl * 3 + k
                blk_re = Mre[r * 128 : (r + 1) * 128, c * 128 : (c + 1) * 128]
                blk_im = Mim[r * 128 : (r + 1) * 128, c * 128 : (c + 1) * 128]
                m_p[:, p * 128 : (p + 1) * 128] = blk_re.T.astype(np.float16)
                m_p[:, (NPAIR + p) * 128 : (NPAIR + p + 1) * 128] = blk_im.T.astype(
                    np.float16
                )

        in_maps.append({"x": x_s, "m": m_p})
    return in_maps


def kernel(x_re, x_im, omega, even_theta, odd_theta, even_phi, odd_phi):
    from concourse.bass_utils import run_bass_kernel_spmd

    in_maps = _host_prepare(
        np.asarray(x_re, np.float32),
        np.asarray(x_im, np.float32),
        np.asarray(omega),
        np.asarray(even_theta),
        np.asarray(odd_theta),
        np.asarray(even_phi),
        np.asarray(odd_phi),
    )
    nc = _build_device_kernel()
    res = run_bass_kernel_spmd(nc, in_maps, core_ids=list(range(NC_CORES)))
    yreT = np.empty((H, B), np.float32)
    yimT = np.empty((H, B), np.float32)
    for core in range(NC_CORES):
        j, i = divmod(core, NI)
        bs = slice(i * BCORE, (i + 1) * BCORE)
        y = res.results[core]["y"]
        for rl in range(RH):
            r = rl if j == 0 else NB - 1 - rl  # mirror for upper-half cores
            rs = slice(r * 128, (r + 1) * 128)
            yreT[rs, bs] = y[rl * 128 : (rl + 1) * 128].astype(np.float32)
            yimT[rs, bs] = y[(RH + rl) * 128 : (RH + rl + 1) * 128].astype(
                np.float32
            )
    out_re = np.ascontiguousarray(yreT.T)
    out_im = np.ascontiguousarray(yimT.T)
    return out_re, out_im



# revision 3
# speedup vs baseline: 1.2830x; 1.2830x over previous
"""EUNN cell (B=2048, H=1024, capacity=128) on 8 NeuronCores.

The 128 Givens layers compose into a banded complex matrix M (bandwidth 128,
block-tridiagonal in 128-blocks); out = D_omega M x. Host composes M (f64) and
quantizes M and x into fp8e4m3 (value, residual) pairs; the device computes the
complex banded matvec with fp8 DoubleRow matmuls (2 K-rows per PE pass, 0.5
cycles/row), accumulating main + x-residual + M-residual terms in fp32 PSUM.
Dropped residual*residual cross terms leave ~9e-3 relative error (gate 2e-2).

Sharding: 8 cores = 4 batch quarters x 2 hidden halves. Per core: 11 real
(row-block, col-block) pairs, 66 DoubleRow matmuls into 8 PSUM banks, outputs
streamed per row-block as fp16.
"""
import numpy as np

H = 1024
B = 2048
CAP = 128
EH = H // 2
OH = (H - 1) // 2
EC = (CAP + 1) // 2
OC = CAP // 2
BAND = CAP
NC_CORES = 8
NB = H // 128          # 8 hidden blocks
NJ = 2                 # hidden halves
NI = 4                 # batch quarters
BCORE = B // NI        # 512 batch cols per core
RH = NB // NJ          # 4 r-blocks per core
CR = RH + 1            # 5 real c-blocks per core (1-block halo)
# (rl, c) pairs per core: rl=0 has 2 cols, rl>=1 have 3
PAIR_COLS = [[0, 1], [0, 1, 2], [1, 2, 3], [2, 3, 4]]
NPAIR = sum(len(c) for c in PAIR_COLS)  # 11
NCOMP = 6              # m comps per pair: [nMia, Mra, Mia, nMib, Mrb, Mib]
NSLAB = 4              # x slabs per c-block: [xia, xra, xib, xrb]

_perm_even = np.arange(EH * 2).reshape(-1, 2)[:, ::-1].reshape(-1)
_perm_odd = np.concatenate(
    [[0], np.arange(1, OH * 2 + 1).reshape(-1, 2)[:, ::-1].reshape(-1), [OH * 2 + 1]]
)


def _interleave(a, b):
    return np.stack([a, b], axis=-1).reshape(-1)


def _layer_coeffs(even_theta, odd_theta, even_phi, odd_phi):
    ce, se = np.cos(even_theta), np.sin(even_theta)
    cpe, spe = np.cos(even_phi), np.sin(even_phi)
    co, so = np.cos(odd_theta), np.sin(odd_theta)
    cpo, spo = np.cos(odd_phi), np.sin(odd_phi)
    zE = np.zeros(EH)
    zO = np.zeros(OH)
    one = np.ones(1)
    zero = np.zeros(1)
    for t in range(EC):
        ect, est, ecp, esp = ce[t], se[t], cpe[t], spe[t]
        v1 = _interleave(esp * ect, ect) + 1j * _interleave(ecp * ect, zE)
        v2 = _interleave(-esp * est, est) + 1j * _interleave(-ecp * est, zE)
        yield v1, v2, _perm_even
        oct_, ost, ocp, osp = co[t], so[t], cpo[t], spo[t]
        v1 = np.concatenate([one, _interleave(osp * oct_, oct_), one]) + 1j * np.concatenate(
            [zero, _interleave(ocp * oct_, zO), zero]
        )
        v2 = np.concatenate([zero, _interleave(-osp * ost, ost), zero]) + 1j * np.concatenate(
            [zero, _interleave(-ocp * ost, zO), zero]
        )
        yield v1, v2, _perm_odd


def _compose_banded(even_theta, odd_theta, even_phi, odd_phi):
    """M = L_128...L_1 as band array bnd[i, d], column j = i + d - BAND."""
    W = 2 * BAND + 1
    bnd = np.zeros((H, W), np.complex64)
    bnd[:, BAND] = 1.0
    new = np.zeros_like(bnd)
    for v1, v2, perm in _layer_coeffs(even_theta, odd_theta, even_phi, odd_phi):
        if perm is _perm_even:
            lo, hi = 0, H
        else:
            lo, hi = 1, H - 1
            new[0] = v1[0] * bnd[0]
            new[H - 1] = v1[H - 1] * bnd[H - 1]
        a = bnd[lo:hi:2]
        b = bnd[lo + 1:hi:2]
        v1a = v1[lo:hi:2, None]
        v2a = v2[lo:hi:2, None]
        v1b = v1[lo + 1:hi:2, None]
        v2b = v2[lo + 1:hi:2, None]
        na = new[lo:hi:2]
        nb = new[lo + 1:hi:2]
        np.multiply(v1a, a, out=na)
        na[:, 1:] += (v2a * b[:, :-1]).astype(np.complex64)
        np.multiply(v1b, b, out=nb)
        nb[:, :-1] += (v2b * a[:, 1:]).astype(np.complex64)
        bnd, new = new, bnd
    return bnd


def _banded_to_dense(bnd):
    M = np.zeros((H, H), bnd.dtype)
    rows = np.arange(H)
    for d in range(2 * BAND + 1):
        j = rows + d - BAND
        ok = (j >= 0) & (j < H)
        M[rows[ok], j[ok]] = bnd[ok, d]
    return M


_NC_CACHE = {}


def _build_device_kernel():
    if "nc" in _NC_CACHE:
        return _NC_CACHE["nc"]
    import concourse.tile as tile
    from concourse import bacc, mybir

    f8 = mybir.dt.float8e4
    f16 = mybir.dt.float16
    f32 = mybir.dt.float32
    DR = mybir.MatmulPerfMode.DoubleRow

    nc = bacc.Bacc("TRN2", target_bir_lowering=False, debug=False)
    x_d = nc.dram_tensor("x", [128, CR * NSLAB * BCORE], f8, kind="ExternalInput").ap()
    m_d = nc.dram_tensor("m", [128, NPAIR * NCOMP * 128], f8, kind="ExternalInput").ap()
    y_d = nc.dram_tensor("y", [128, RH * 2 * BCORE], f16, kind="ExternalOutput").ap()

    x_v = x_d.rearrange("p (c s b) -> p c s b", c=CR, s=NSLAB)
    m_v = m_d.rearrange("p (q n k) -> p q n k", q=NPAIR, n=NCOMP)
    y_v = y_d.rearrange("p (r s b) -> p r s b", r=RH, s=2)

    # m chunk per rl (pair ranges), x chunk per c-block; interleaved so the
    # first psum group unblocks as early as possible.
    m_ranges = []
    p0 = 0
    for cols in PAIR_COLS:
        m_ranges.append((p0, p0 + len(cols)))
        p0 += len(cols)

    with tile.TileContext(nc) as tc:
        with (
            tc.tile_pool(name="mp", bufs=1) as mpool,
            tc.tile_pool(name="xp", bufs=1) as xpool,
            tc.tile_pool(name="op", bufs=1) as opool,
            tc.tile_pool(name="pp", bufs=1, space="PSUM") as pspool,
        ):
            m_t = mpool.tile([128, NPAIR * NCOMP * 128], f8, tag="m")
            x_t = xpool.tile([128, CR * NSLAB * BCORE], f8, tag="x")
            y_t = opool.tile([128, RH * 2 * BCORE], f16, tag="y")
            m_r = m_t[:].rearrange("p (q n k) -> p q n k", q=NPAIR, n=NCOMP)
            x_r = x_t[:].rearrange("p (c s b) -> p c s b", c=CR, s=NSLAB)
            y_r = y_t[:].rearrange("p (r s b) -> p r s b", r=RH, s=2)

            # interleaved input DMA schedule: m_rl0, x0, x1, m_rl1, x2, ...
            for kind, idx in (
                ("m", 0), ("x", 0), ("x", 1), ("m", 1),
                ("x", 2), ("m", 2), ("x", 3), ("m", 3), ("x", 4),
            ):
                if kind == "m":
                    a, b = m_ranges[idx]
                    nc.sync.dma_start(m_r[:, a:b], m_v[:, a:b])
                else:
                    nc.sync.dma_start(x_r[:, idx], x_v[:, idx])

            psr = [pspool.tile([128, BCORE], f32, tag=f"psr{r}", name=f"psr{r}")
                   for r in range(RH)]
            psi = [pspool.tile([128, BCORE], f32, tag=f"psi{r}", name=f"psi{r}")
                   for r in range(RH)]

            for rl in range(RH):
                cols = PAIR_COLS[rl]
                a, _ = m_ranges[rl]
                n = len(cols)
                for k, c in enumerate(cols):
                    p = a + k
                    first = k == 0
                    last = k == n - 1
                    # rhs pairs: lo = (xia, xra), hi = (xib, xrb)
                    rlo = x_r[:, c, 0:2]
                    rhi = x_r[:, c, 2:4]
                    # lhsT pairs within comps [nMia, Mra, Mia, nMib, Mrb, Mib]
                    pr_m = m_r[:, p, 0:2]   # (nMia, Mra): psr main/xres
                    pi_m = m_r[:, p, 1:3]   # (Mra, Mia):  psi main/xres
                    pr_r = m_r[:, p, 3:5]   # (nMib, Mrb): psr Mres
                    pi_r = m_r[:, p, 4:6]   # (Mrb, Mib):  psi Mres
                    nc.tensor.matmul(psr[rl][:], lhsT=pr_m, rhs=rlo,
                                     start=first, stop=False, perf_mode=DR)
                    nc.tensor.matmul(psi[rl][:], lhsT=pi_m, rhs=rlo,
                                     start=first, stop=False, perf_mode=DR)
                    nc.tensor.matmul(psr[rl][:], lhsT=pr_m, rhs=rhi,
                                     start=False, stop=False, perf_mode=DR)
                    nc.tensor.matmul(psi[rl][:], lhsT=pi_m, rhs=rhi,
                                     start=False, stop=False, perf_mode=DR)
                    nc.tensor.matmul(psr[rl][:], lhsT=pr_r, rhs=rlo,
                                     start=False, stop=last, perf_mode=DR)
                    nc.tensor.matmul(psi[rl][:], lhsT=pi_r, rhs=rlo,
                                     start=False, stop=last, perf_mode=DR)
                # PSUM -> SBUF fp16, split across DVE and Act engines
                nc.vector.tensor_copy(y_r[:, rl, 0], psr[rl][:])
                nc.scalar.copy(y_r[:, rl, 1], psi[rl][:])

            for rl in range(RH):
                nc.sync.dma_start(y_v[:, rl], y_r[:, rl])

    nc.compile()
    _NC_CACHE["nc"] = nc
    return nc


def _host_prepare(x_re, x_im, omega, even_theta, odd_theta, even_phi, odd_phi):
    """Compose M, fold omega, quantize to fp8 (value, residual) pairs, pack."""
    import ml_dtypes

    F8 = ml_dtypes.float8_e4m3

    def q8(a):
        return np.asarray(a, np.float32).astype(F8)

    bnd = _compose_banded(
        even_theta.astype(np.float64),
        odd_theta.astype(np.float64),
        even_phi.astype(np.float64),
        odd_phi.astype(np.float64),
    )
    M = _banded_to_dense(bnd)
    w = omega.astype(np.float64)
    Mw = (np.cos(w) + 1j * np.sin(w))[:, None] * M
    Mre = np.asarray(Mw.real, np.float32)
    Mim = np.asarray(Mw.imag, np.float32)
    Mra = q8(Mre)
    Mrb = q8(Mre - Mra.astype(np.float32))
    Mia = q8(Mim)
    Mib = q8(Mim - Mia.astype(np.float32))

    xrT = np.ascontiguousarray(x_re.T).astype(np.float32)  # [H, B]
    xiT = np.ascontiguousarray(x_im.T).astype(np.float32)
    XRA = q8(xrT)
    XRB = q8(xrT - XRA.astype(np.float32))
    XIA = q8(xiT)
    XIB = q8(xiT - XIA.astype(np.float32))

    # m packs per hidden half (shared by the 4 batch quarters)
    m_packs = []
    for j in range(NJ):
        m_p = np.zeros((128, NPAIR, NCOMP, 128), F8)
        p = 0
        for rl in range(RH):
            r = rl if j == 0 else NB - 1 - rl
            for cl in PAIR_COLS[rl]:
                c = cl if j == 0 else NB - 1 - cl
                rs = slice(r * 128, (r + 1) * 128)
                cs = slice(c * 128, (c + 1) * 128)
                # lhsT: [K = c rows, out = r cols]
                m_p[:, p, 1] = Mra[rs, cs].T
                m_p[:, p, 2] = Mia[rs, cs].T
                m_p[:, p, 4] = Mrb[rs, cs].T
                m_p[:, p, 5] = Mib[rs, cs].T
                m_p[:, p, 0] = -m_p[:, p, 2]  # nMia
                m_p[:, p, 3] = -m_p[:, p, 5]  # nMib
                p += 1
        m_packs.append(np.ascontiguousarray(m_p.reshape(128, -1)))

    in_maps = []
    for core in range(NC_CORES):
        j, i = divmod(core, NI)
        bs = slice(i * BCORE, (i + 1) * BCORE)
        x_s = np.empty((128, CR, NSLAB, BCORE), F8)
        for s in range(CR):
            g = s if j == 0 else NB - 1 - s
            gs = slice(g * 128, (g + 1) * 128)
            x_s[:, s, 0] = XIA[gs, bs]
            x_s[:, s, 1] = XRA[gs, bs]
            x_s[:, s, 2] = XIB[gs, bs]
            x_s[:, s, 3] = XRB[gs, bs]
        in_maps.append({"x": np.ascontiguousarray(x_s.reshape(128, -1)),
                        "m": m_packs[j]})
    return in_maps


def kernel(x_re, x_im, omega, even_theta, odd_theta, even_phi, odd_phi):
    from concourse.bass_utils import run_bass_kernel_spmd

    in_maps = _host_prepare(
        np.asarray(x_re, np.float32),
        np.asarray(x_im, np.float32),
        np.asarray(omega),
        np.asarray(even_theta),
        np.asarray(odd_theta),
        np.asarray(even_phi),
        np.asarray(odd_phi),
    )
    nc = _build_device_kernel()
    res = run_bass_kernel_spmd(nc, in_maps, core_ids=list(range(NC_CORES)))
    yreT = np.empty((H, B), np.float32)
    yimT = np.empty((H, B), np.float32)
    for core in range(NC_CORES):
        j, i = divmod(core, NI)
        bs = slice(i * BCORE, (i + 1) * BCORE)
        y = res.results[core]["y"].reshape(128, RH, 2, BCORE)
        for rl in range(RH):
            r = rl if j == 0 else NB - 1 - rl
            rs = slice(r * 128, (r + 1) * 128)
            yreT[rs, bs] = y[:, rl, 0].astype(np.float32)
            yimT[rs, bs] = y[:, rl, 1].astype(np.float32)
    out_re = np.ascontiguousarray(yreT.T)
    out_im = np.ascontiguousarray(yimT.T)
    return out_re, out_im
